# revision 13
# baseline (speedup 1.0000x reference)
"""Trainium2 Bass kernel for nn_AttenuationToRainRate (dense_mlp).

Reference computation per (sample b, position t):
  style MLP: metadata (16) -> 64 -> 128 -> 64, split into 4 x (scale, bias)[8]
  main chain: x -> [w1] -> adain/lrelu -> [w2] -> adain/lrelu -> [w3] ->
              adain/lrelu -> [w4] -> adain/lrelu -> [w5] -> lrelu
  adain(h) = scale * (h - mean_c h) / (std_ddof1(h) + 1e-6) + bias

Design (v4): data-parallel over 8 cores (32 samples each).  Layout: tile
[128, N] with partition p = 8*s' + c (16 samples x 8 channels), free =
positions.  Mean-removal folded into weights host-side (W' = W(I-J/8),
b' = b - mean b).  Per layer l, per 2048-position block, engine split:
    d0  : PE matmul (block-diag W', no bias pass)      -> PSUM [128,1024]x2
    dcp : d0 + b'  (bias folded into the PSUM->SBUF extraction)
          L1 on ACT Identity (bias inside w1aug), L2-4 on DVE tensor_scalar
    dsq : DVE tensor_tensor(dcp, dcp)  fp16 SBUF (2x mode)
    vb  : PE matmul block-ones @ dsq                   -> PSUM [128,1024]x2
    r   : ACT Abs_reciprocal_sqrt(vb/7 + eps)  -> fp16 SBUF
    q   : DVE tensor_tensor(dcp, r)
    a   : ACT Prelu(scv*q + bcv, alpha=.01)  (affine fused in ACT)
  Layer 5 (no adain): banded stationaries into packed [64,512] PSUM,
  out = Prelu(h5 + b5), DMA'd with a rearrange that undoes the packing.
  PSUM is one 8-bank pool ([128,1024] x 4 bufs) rotating d-tiles and
  vb-tiles so consecutive layers/blocks pipeline.
  All ACT funcs used live in one activation-table set, pinned via a Bacc
  subclass so the table is loaded exactly once.
"""

import numpy as np

B_FULL, T = 256, 8192
NCORES = 8
BS = B_FULL // NCORES  # 32 samples per core
F = 16

# config switches (test.py may flip these and call _reset())
CFG = {
    "mm_dt": "fp16",   # matmul operand dtype
    "dcp_eng": "dve",  # L2-4 extraction engine: dve | act
    "prelu_eng": "act",  # a = prelu(...): act | dve
}

_CACHE = {}


def _reset():
    _CACHE.clear()


# ----------------------------------------------------------------- host side

def _host_weights(inp):
    """Weight-derived constants in device layouts (f32 numpy)."""
    f64 = np.float64
    I8 = np.eye(8, dtype=f64)
    C = I8 - np.full((8, 8), 1.0 / 8.0, dtype=f64)  # output-centering

    w = {}
    w1 = np.asarray(inp["w1"], dtype=f64)           # (1, 8)
    b1 = np.asarray(inp["b1"], dtype=f64)           # (8,)
    w1p = (w1 @ C)[0]
    b1p = b1 - b1.mean()
    w1aug = np.zeros((17, 128), dtype=f64)
    for s in range(16):
        w1aug[s, 8 * s:8 * s + 8] = w1p
        w1aug[16, 8 * s:8 * s + 8] = b1p
    w["w1aug"] = w1aug

    for l in (2, 3, 4):
        W = np.asarray(inp[f"w{l}"], dtype=f64) @ C
        bp = np.asarray(inp[f"b{l}"], dtype=f64)
        bp = bp - bp.mean()
        wb = np.zeros((128, 128), dtype=f64)
        for s in range(16):
            wb[8 * s:8 * s + 8, 8 * s:8 * s + 8] = W
        w[f"wb{l}"] = wb
        w[f"bv{l}"] = np.tile(bp, 16).reshape(128, 1)  # f32 bias vector

    b8 = np.zeros((128, 128), dtype=f64)
    for s in range(16):
        b8[8 * s:8 * s + 8, 8 * s:8 * s + 8] = 1.0
    w["b8bc"] = b8

    # packed channel-sum: vbp[32*tau+s, n] = sum_c dsq[8s+c, n];
    # cols 16-31 zero so matmul outputs land on 32-partition boundaries
    o32 = np.zeros((128, 32), dtype=f64)
    for s in range(16):
        o32[8 * s:8 * s + 8, s] = 1.0
    w["ones32"] = o32

    w5b = np.zeros((128, 4 * 64), dtype=f64)
    w5 = np.asarray(inp["w5"], dtype=f64)[:, 0]
    for tau in range(4):
        for s in range(16):
            for c in range(8):
                w5b[8 * s + c, 64 * tau + 16 * tau + s] = w5[c]
    w["w5b"] = w5b
    w["b5c"] = np.full((64, 1), float(np.asarray(inp["b5"], dtype=f64)[0]))

    w["onesr"] = np.ones((1, 2048), dtype=f64)
    w["mw1"] = np.asarray(inp["mw1"], dtype=f64)
    w["mw2"] = np.asarray(inp["mw2"], dtype=f64)
    w["mw3"] = np.asarray(inp["mw3"], dtype=f64)
    w["mb1c"] = np.asarray(inp["mb1"], dtype=f64).reshape(64, 1)
    w["mb2c"] = np.asarray(inp["mb2"], dtype=f64).reshape(128, 1)
    w["mb3c"] = np.asarray(inp["mb3"], dtype=f64).reshape(64, 1)

    mm_np = {"fp16": np.float16, "f32": np.float32}[CFG["mm_dt"]]
    out = {}
    for k, v in w.items():
        dt = mm_np if k in _MM_STAT else np.float32
        out[k] = np.ascontiguousarray(v.astype(dt))
    return out


_WSHAPES = {
    "w1aug": [17, 128],
    "wb2": [128, 128], "wb3": [128, 128], "wb4": [128, 128],
    "bv2": [128, 1], "bv3": [128, 1], "bv4": [128, 1],
    "b8bc": [128, 128], "ones32": [128, 32],
    "w5b": [128, 256], "b5c": [64, 1],
    "onesr": [1, 2048],
    "mw1": [16, 64], "mw2": [64, 128], "mw3": [128, 64],
    "mb1c": [64, 1], "mb2c": [128, 1], "mb3c": [64, 1],
}
# tensors that feed PE matmuls (get the matmul dtype)
_MM_STAT = {"w1aug", "wb2", "wb3", "wb4", "b8bc", "ones32", "w5b", "onesr"}


# --------------------------------------------------------------- device side

def build_program(cfg=None):
    import concourse.bacc as bacc
    import concourse.mybir as mybir
    from concourse.ap import AP
    from concourse.tile import TileContext

    cfg = dict(CFG if cfg is None else cfg)
    f32 = mybir.dt.float32
    f16 = mybir.dt.float16
    mdt = {"fp16": f16, "f32": f32}[cfg["mm_dt"]]
    AF = mybir.ActivationFunctionType
    OP = mybir.AluOpType

    class _KBacc(bacc.Bacc):
        # Pin all activation functions to one table set to avoid
        # ACT_TABLE_LOAD churn (see v3 docstring).
        _ACT_SET = "abs_reciprocal_sqrt_and_small"

        def insert_act_table_loads(self):
            import concourse.mybir as _mb
            from concourse.hw_specs import get_activation_tables
            has_activation = any(
                isinstance(i, _mb.InstActivation)
                for b in self.main_func.blocks
                for i in b.instructions
            )
            if not has_activation:
                return
            tables = []
            for name, funcs in get_activation_tables(self.m.arch).items():
                tables.append((name, funcs if name == self._ACT_SET else set()))
            bacc._bass_rust.insert_act_table_loads(self, tables)

    nc = _KBacc("TRN2", target_bir_lowering=False)
    x_d = nc.dram_tensor("x", [BS, T], mdt, kind="ExternalInput")
    md_d = nc.dram_tensor("metadata", [BS, F], f32, kind="ExternalInput")
    y_d = nc.dram_tensor("y", [BS, T], f32, kind="ExternalOutput")
    wd = {name: nc.dram_tensor(name, shp, mdt if name in _MM_STAT else f32,
                               kind="ExternalInput")
          for name, shp in _WSHAPES.items()}

    with TileContext(nc) as tc:
        with tc.tile_pool(name="const", bufs=1) as cp, \
             tc.tile_pool(name="scr", bufs=1, space="DRAM") as dp:

            # ---- constants to SBUF
            cw = {}
            for name, shp in _WSHAPES.items():
                t = cp.tile(shp, mdt if name in _MM_STAT else f32,
                            name=f"c_{name}")
                nc.sync.dma_start(out=t[:], in_=wd[name][:])
                cw[name] = t
            eps_s = cp.tile([128, 1], f32, name="eps_s")
            nc.vector.memset(eps_s[:], 1e-12)

            # ---- style MLP (per-core 32 samples)
            with tc.tile_pool(name="stp", bufs=1, space="PSUM") as sp:
                mdT = cp.tile([F, BS], f32, name="mdT")
                nc.sync.dma_start(out=mdT[:], in_=md_d.rearrange("s f -> f s"))
                ps1 = sp.tile([64, BS], f32, name="ps1")
                nc.tensor.matmul(ps1[:], cw["mw1"][:], mdT[:],
                                 start=True, stop=True)
                s1 = cp.tile([64, BS], f32, name="s1")
                nc.scalar.activation(s1[:], ps1[:], AF.Relu, bias=cw["mb1c"][:])
                ps2 = sp.tile([128, BS], f32, name="ps2")
                nc.tensor.matmul(ps2[:], cw["mw2"][:], s1[:],
                                 start=True, stop=True)
                s2 = cp.tile([128, BS], f32, name="s2")
                nc.scalar.activation(s2[:], ps2[:], AF.Relu, bias=cw["mb2c"][:])
                ps3 = sp.tile([64, BS], f32, name="ps3")
                nc.tensor.matmul(ps3[:], cw["mw3"][:], s2[:],
                                 start=True, stop=True)
                sT = cp.tile([64, BS], f32, name="sT")
                nc.scalar.activation(sT[:], ps3[:], AF.Identity,
                                     bias=cw["mb3c"][:])

            # ---- per-(layer, supergroup) scale/bias vectors via DRAM trip
            # sT row = 16(l-1) + 2c + (0 scale / 1 bias), col = 16 sg + s'
            sT_d = dp.tile([64, BS], f32, name="sT_d")
            nc.gpsimd.dma_start(out=sT_d[:], in_=sT[:])
            scv = cp.tile([128, 8], f32, name="scv")   # scale, col j=(l-1)*2+sg
            bcv = cp.tile([128, 8], f32, name="bcv")   # bias
            for l in range(1, 5):
                for g in range(2):
                    j = (l - 1) * 2 + g
                    src_s = AP(tensor=sT_d[:].tensor,
                               offset=512 * (l - 1) + 16 * g,
                               ap=((1, 16), (64, 8)))
                    nc.gpsimd.dma_start(out=scv[:, j:j + 1], in_=src_s)
                    src_b = AP(tensor=sT_d[:].tensor,
                               offset=512 * (l - 1) + 32 + 16 * g,
                               ap=((1, 16), (64, 8)))
                    nc.gpsimd.dma_start(out=bcv[:, j:j + 1], in_=src_b)

            # ---------------- main loop
            # Two PSUM pools: "pd" holds d-tiles (+ L5 h5), "pv" holds var
            # tiles; bufs=2 each (8 banks total) so consecutive blocks
            # overlap.  Each 2048-position block is processed as two
            # independent 1024-position streams (c=0,1) that alternate
            # engines: c0 extracts on ACT / prelus on DVE, c1 the opposite.
            with tc.tile_pool(name="pd", bufs=3, space="PSUM") as pdp, \
                 tc.tile_pool(name="pv", bufs=2, space="PSUM") as pvp, \
                 tc.tile_pool(name="xin", bufs=5) as xp, \
                 tc.tile_pool(name="dcpp", bufs=4) as dcpp, \
                 tc.tile_pool(name="dsqp", bufs=3) as dqp, \
                 tc.tile_pool(name="rpk", bufs=3) as rkp, \
                 tc.tile_pool(name="rbc", bufs=3) as rpp, \
                 tc.tile_pool(name="qp", bufs=3) as qpp, \
                 tc.tile_pool(name="ap", bufs=4) as app, \
                 tc.tile_pool(name="outp", bufs=3) as opp:

                def make_stream(g, k):
                    """Return the list of phase closures for block (g, k).

                    27 phases: [xt] + 4 layers x [dmm, extract, vmm, r, q,
                    prelu] + [L5 mm, out].  State is shared via `st`.
                    """
                    st = {}
                    phases = []

                    def p_xt():
                        xt = xp.tile([17, 2048], mdt, name="xt", tag="xt")
                        nc.sync.dma_start(
                            out=xt[0:16, :],
                            in_=x_d[16 * g:16 * g + 16,
                                    2048 * k:2048 * (k + 1)])
                        nc.sync.dma_start(out=xt[16:17, :],
                                          in_=cw["onesr"][:])
                        st["xt"] = xt
                    phases.append(p_xt)

                    def mk_dmm(l):
                        def p_dmm():
                            dts = [pdp.tile([128, 1024], f32,
                                            name=f"dt{l}{c}", tag="pt")
                                   for c in range(2)]
                            src = st["xt"] if l == 1 else st["a"]
                            wname = "w1aug" if l == 1 else f"wb{l}"
                            for tau in range(4):
                                dst = dts[tau // 2][:, 512 * (tau % 2):
                                                    512 * (tau % 2) + 512]
                                sl = slice(512 * tau, 512 * (tau + 1))
                                nc.tensor.matmul(dst, cw[wname][:],
                                                 src[:, sl],
                                                 start=True, stop=True)
                            st["dts"] = dts
                        return p_dmm

                    def mk_ext(l):
                        def p_ext():
                            dts = st["dts"]
                            dcp = dcpp.tile([128, 2048], f16,
                                            name=f"dcp{l}", tag="dcp")
                            dsq = dqp.tile([128, 2048], f16,
                                           name=f"dsq{l}", tag="dsq")
                            bias_ap = (None if l == 1
                                       else cw[f"bv{l}"][:, 0:1])
                            # c0 on ACT
                            if bias_ap is None:
                                nc.scalar.activation(dcp[:, 0:1024],
                                                     dts[0][:], AF.Identity)
                            else:
                                nc.scalar.activation(dcp[:, 0:1024],
                                                     dts[0][:], AF.Identity,
                                                     bias=bias_ap)
                            # c1 on DVE, then both squares on DVE
                            nc.vector.tensor_scalar(
                                dcp[:, 1024:2048], dts[1][:],
                                0.0 if bias_ap is None else bias_ap,
                                None, OP.add)
                            nc.vector.tensor_tensor(
                                dsq[:, 1024:2048], dcp[:, 1024:2048],
                                dcp[:, 1024:2048], OP.mult)
                            nc.vector.tensor_tensor(
                                dsq[:, 0:1024], dcp[:, 0:1024],
                                dcp[:, 0:1024], OP.mult)
                            st["dcp"], st["dsq"] = dcp, dsq
                        return p_ext

                    def mk_vmm(l):
                        def p_vmm():
                            dsq = st["dsq"]
                            # packed: vbp[32*tau+s, n] = sum_c dsq[8s+c, ...]
                            vbp = pvp.tile([128, 512], f32,
                                           name=f"vb{l}", tag="pv")
                            for tau in range(4):
                                sl = slice(512 * tau, 512 * (tau + 1))
                                nc.tensor.matmul(
                                    vbp[32 * tau:32 * tau + 32, :],
                                    cw["ones32"][:], dsq[:, sl],
                                    start=True, stop=True,
                                    tile_position=(0, 32 * tau))
                            st["vbp"] = vbp
                        return p_vmm

                    def mk_r(l):
                        def p_r():
                            # packed rsqrt, then DMA partition-broadcast to
                            # the full [128, 2048] layout
                            rp_s = rkp.tile([128, 512], f16,
                                            name=f"rp{l}", tag="rp")
                            nc.scalar.activation(
                                rp_s[:], st["vbp"][:],
                                AF.Abs_reciprocal_sqrt,
                                scale=1.0 / 7.0, bias=eps_s[:])
                            rbc = rpp.tile([128, 2048], f16,
                                           name=f"rbc{l}", tag="rbc")
                            sub = rp_s[:]
                            for tau in range(4):
                                src = AP(tensor=sub.tensor,
                                         offset=sub.offset + 16384 * tau,
                                         ap=[[512, 16], [0, 8], [1, 512]])
                                nc.sync.dma_start(
                                    out=rbc[:, 512 * tau:512 * (tau + 1)],
                                    in_=src)
                            st["r"] = rbc
                        return p_r

                    def mk_q(l):
                        def p_q():
                            q_ = qpp.tile([128, 2048], f16,
                                          name=f"q{l}", tag="q")
                            for c in range(2):
                                csl = slice(1024 * c, 1024 * (c + 1))
                                nc.vector.tensor_tensor(
                                    q_[:, csl], st["dcp"][:, csl],
                                    st["r"][:, csl], OP.mult)
                            st["q"] = q_
                        return p_q

                    def mk_prelu(l):
                        j = (l - 1) * 2 + g

                        def p_prelu():
                            q_ = st["q"]
                            anew = app.tile([128, 2048], mdt,
                                            name=f"a{l}", tag="a")
                            for c in range(2):
                                csl = slice(1024 * c, 1024 * (c + 1))
                                nc.scalar.activation(
                                    anew[:, csl], q_[:, csl],
                                    AF.Prelu, scale=scv[:, j:j + 1],
                                    bias=bcv[:, j:j + 1], alpha=0.01)
                            st["a"] = anew
                        return p_prelu

                    for l in range(1, 5):
                        phases += [mk_dmm(l), mk_ext(l), mk_vmm(l),
                                   mk_r(l), mk_q(l), mk_prelu(l)]

                    def p_l5mm():
                        h5t = pdp.tile([128, 1024], f32, name="h5",
                                       tag="pt")
                        h5 = h5t[0:64, 0:512]
                        a4 = st["a"]
                        for tau in range(4):
                            sl = slice(512 * tau, 512 * (tau + 1))
                            nc.tensor.matmul(
                                h5, cw["w5b"][:, 64 * tau:64 * (tau + 1)],
                                a4[:, sl],
                                start=(tau == 0), stop=(tau == 3))
                        st["h5"] = h5
                    phases.append(p_l5mm)

                    def p_out():
                        oc = opp.tile([64, 512], f32, name="oc", tag="oc")
                        nc.scalar.activation(oc[:], st["h5"], AF.Prelu,
                                             bias=cw["b5c"][:], alpha=0.01)
                        ydst = y_d.rearrange(
                            "(sg sp) (kk tau n) -> sg kk tau sp n",
                            sg=2, kk=4, tau=4, n=512)[g, k]
                        # oc partition-major order (p = 16 tau + sp) matches
                        # the (tau, sp, n) iteration of ydst
                        nc.sync.dma_start(out=ydst, in_=oc[:])
                    phases.append(p_out)
                    return phases

                # software-pipelined wavefront emission: streams start
                # STAG phases apart; most-advanced stream emitted first
                streams = [make_stream(g, k)
                           for g in range(2) for k in range(4)]
                STAG = int(cfg.get("stag", 7))
                nph = len(streams[0])
                total = STAG * (len(streams) - 1) + nph
                for t in range(total):
                    for s, phases in enumerate(streams):
                        p = t - STAG * s
                        if 0 <= p < nph:
                            phases[p]()

    nc.compile()
    return nc


# ------------------------------------------------------------------- runner

def _get_program():
    key = tuple(sorted(CFG.items()))
    if key not in _CACHE:
        _CACHE[key] = build_program(CFG)
    return _CACHE[key]


def _make_in_maps(inputs):
    mm_np = {"fp16": np.float16, "f32": np.float32}[CFG["mm_dt"]]
    x = np.ascontiguousarray(
        np.asarray(inputs["x"], dtype=np.float32).reshape(B_FULL, T).astype(
            mm_np))
    md = np.ascontiguousarray(np.asarray(inputs["metadata"], dtype=np.float32))
    wts = _host_weights(inputs)
    in_maps = []
    for i in range(NCORES):
        m = dict(wts)
        m["x"] = np.ascontiguousarray(x[BS * i:BS * (i + 1)])
        m["metadata"] = np.ascontiguousarray(md[BS * i:BS * (i + 1)])
        in_maps.append(m)
    return in_maps


def run_spmd(inputs, trace=False):
    """Run on all 8 cores; returns (y_full, BassKernelResults)."""
    from concourse.bass_utils import run_bass_kernel_spmd
    nc = _get_program()
    in_maps = _make_in_maps(inputs)
    res = run_bass_kernel_spmd(nc, in_maps, core_ids=list(range(NCORES)),
                               trace=trace)
    y = np.concatenate([np.asarray(r["y"]) for r in res.results], axis=0)
    y = y.reshape(B_FULL, 1, T).astype(np.float32)
    return y, res


def kernel(**inputs):
    y, _ = run_spmd(inputs, trace=False)
    return y


# revision 16
# speedup vs baseline: 1.1632x; 1.1632x over previous
"""Trainium2 Bass kernel for nn_AttenuationToRainRate (dense_mlp).

Reference computation per (sample b, position t):
  style MLP: metadata (16) -> 64 -> 128 -> 64, split into 4 x (scale, bias)[8]
  main chain: x -> [w1] -> adain/lrelu -> [w2] -> adain/lrelu -> [w3] ->
              adain/lrelu -> [w4] -> adain/lrelu -> [w5] -> lrelu
  adain(h) = scale * (h - mean_c h) / (std_ddof1(h) + 1e-6) + bias

Design (v4): data-parallel over 8 cores (32 samples each).  Layout: tile
[128, N] with partition p = 8*s' + c (16 samples x 8 channels), free =
positions.  Mean-removal folded into weights host-side (W' = W(I-J/8),
b' = b - mean b).  Per layer l, per 2048-position block, engine split:
    d0  : PE matmul (block-diag W', no bias pass)      -> PSUM [128,1024]x2
    dcp : d0 + b'  (bias folded into the PSUM->SBUF extraction)
          L1 on ACT Identity (bias inside w1aug), L2-4 on DVE tensor_scalar
    dsq : DVE tensor_tensor(dcp, dcp)  fp16 SBUF (2x mode)
    vb  : PE matmul block-ones @ dsq                   -> PSUM [128,1024]x2
    r   : ACT Abs_reciprocal_sqrt(vb/7 + eps)  -> fp16 SBUF
    q   : DVE tensor_tensor(dcp, r)
    a   : ACT Prelu(scv*q + bcv, alpha=.01)  (affine fused in ACT)
  Layer 5 (no adain): banded stationaries into packed [64,512] PSUM,
  out = Prelu(h5 + b5), DMA'd with a rearrange that undoes the packing.
  PSUM is one 8-bank pool ([128,1024] x 4 bufs) rotating d-tiles and
  vb-tiles so consecutive layers/blocks pipeline.
  All ACT funcs used live in one activation-table set, pinned via a Bacc
  subclass so the table is loaded exactly once.
"""

import numpy as np

B_FULL, T = 256, 8192
NCORES = 8
BS = B_FULL // NCORES  # 32 samples per core
F = 16

# config switches (test.py may flip these and call _reset())
CFG = {
    "mm_dt": "fp16",   # matmul operand dtype
    "dcp_eng": "dve",  # L2-4 extraction engine: dve | act
    "prelu_eng": "act",  # a = prelu(...): act | dve
}

_CACHE = {}


def _reset():
    _CACHE.clear()


# ----------------------------------------------------------------- host side

def _host_weights(inp):
    """Weight-derived constants in device layouts (f32 numpy)."""
    f64 = np.float64
    I8 = np.eye(8, dtype=f64)
    C = I8 - np.full((8, 8), 1.0 / 8.0, dtype=f64)  # output-centering

    w = {}
    w1 = np.asarray(inp["w1"], dtype=f64)           # (1, 8)
    b1 = np.asarray(inp["b1"], dtype=f64)           # (8,)
    w1p = (w1 @ C)[0]
    b1p = b1 - b1.mean()
    w1aug = np.zeros((17, 128), dtype=f64)
    for s in range(16):
        w1aug[s, 8 * s:8 * s + 8] = w1p
        w1aug[16, 8 * s:8 * s + 8] = b1p
    w["w1aug"] = w1aug

    for l in (2, 3, 4):
        W = np.asarray(inp[f"w{l}"], dtype=f64) @ C
        bp = np.asarray(inp[f"b{l}"], dtype=f64)
        bp = bp - bp.mean()
        wb = np.zeros((128, 128), dtype=f64)
        for s in range(16):
            wb[8 * s:8 * s + 8, 8 * s:8 * s + 8] = W
        w[f"wb{l}"] = wb
        w[f"bv{l}"] = np.tile(bp, 16).reshape(128, 1)  # f32 bias vector

    b8 = np.zeros((128, 128), dtype=f64)
    for s in range(16):
        b8[8 * s:8 * s + 8, 8 * s:8 * s + 8] = 1.0
    w["b8bc"] = b8

    # packed channel-sum: vbp[32*tau+s, n] = sum_c dsq[8s+c, n];
    # cols 16-31 zero so matmul outputs land on 32-partition boundaries
    o32 = np.zeros((128, 32), dtype=f64)
    for s in range(16):
        o32[8 * s:8 * s + 8, s] = 1.0
    w["ones32"] = o32

    # r-broadcast stationaries: rbc[8s+c, n] = rp[32*tau+s, n]
    ebc = np.zeros((128, 4 * 128), dtype=f64)
    for tau in range(4):
        for s in range(16):
            for c in range(8):
                ebc[32 * tau + s, 128 * tau + 8 * s + c] = 1.0
    w["ebc"] = ebc

    w5b = np.zeros((128, 4 * 64), dtype=f64)
    w5 = np.asarray(inp["w5"], dtype=f64)[:, 0]
    for tau in range(4):
        for s in range(16):
            for c in range(8):
                w5b[8 * s + c, 64 * tau + 16 * tau + s] = w5[c]
    w["w5b"] = w5b
    w["b5c"] = np.full((64, 1), float(np.asarray(inp["b5"], dtype=f64)[0]))

    w["onesr"] = np.ones((1, 2048), dtype=f64)
    # eps bias for packed rsqrt: pad rows (16-31 of each 32-group) get 1.0
    # so their fp16 rsqrt output stays finite (never read with weight != 0)
    ev = np.full((128, 1), 1e-12, dtype=f64)
    for g32 in range(4):
        ev[32 * g32 + 16:32 * g32 + 32] = 1.0
    w["epsv"] = ev
    w["mw1"] = np.asarray(inp["mw1"], dtype=f64)
    w["mw2"] = np.asarray(inp["mw2"], dtype=f64)
    w["mw3"] = np.asarray(inp["mw3"], dtype=f64)
    w["mb1c"] = np.asarray(inp["mb1"], dtype=f64).reshape(64, 1)
    w["mb2c"] = np.asarray(inp["mb2"], dtype=f64).reshape(128, 1)
    w["mb3c"] = np.asarray(inp["mb3"], dtype=f64).reshape(64, 1)

    mm_np = {"fp16": np.float16, "f32": np.float32}[CFG["mm_dt"]]
    out = {}
    for k, v in w.items():
        dt = mm_np if k in _MM_STAT else np.float32
        out[k] = np.ascontiguousarray(v.astype(dt))
    return out


_WSHAPES = {
    "w1aug": [17, 128],
    "wb2": [128, 128], "wb3": [128, 128], "wb4": [128, 128],
    "bv2": [128, 1], "bv3": [128, 1], "bv4": [128, 1],
    "b8bc": [128, 128], "ones32": [128, 32], "ebc": [128, 512],
    "w5b": [128, 256], "b5c": [64, 1],
    "onesr": [1, 2048], "epsv": [128, 1],
    "mw1": [16, 64], "mw2": [64, 128], "mw3": [128, 64],
    "mb1c": [64, 1], "mb2c": [128, 1], "mb3c": [64, 1],
}
# tensors that feed PE matmuls (get the matmul dtype)
_MM_STAT = {"w1aug", "wb2", "wb3", "wb4", "b8bc", "ones32", "ebc", "w5b",
            "onesr"}


# --------------------------------------------------------------- device side

def build_program(cfg=None):
    import concourse.bacc as bacc
    import concourse.mybir as mybir
    from concourse.ap import AP
    from concourse.tile import TileContext

    cfg = dict(CFG if cfg is None else cfg)
    f32 = mybir.dt.float32
    f16 = mybir.dt.float16
    mdt = {"fp16": f16, "f32": f32}[cfg["mm_dt"]]
    AF = mybir.ActivationFunctionType
    OP = mybir.AluOpType

    class _KBacc(bacc.Bacc):
        # Pin all activation functions to one table set to avoid
        # ACT_TABLE_LOAD churn (see v3 docstring).
        _ACT_SET = "abs_reciprocal_sqrt_and_small"

        def insert_act_table_loads(self):
            import concourse.mybir as _mb
            from concourse.hw_specs import get_activation_tables
            has_activation = any(
                isinstance(i, _mb.InstActivation)
                for b in self.main_func.blocks
                for i in b.instructions
            )
            if not has_activation:
                return
            tables = []
            for name, funcs in get_activation_tables(self.m.arch).items():
                tables.append((name, funcs if name == self._ACT_SET else set()))
            bacc._bass_rust.insert_act_table_loads(self, tables)

    nc = _KBacc("TRN2", target_bir_lowering=False)
    x_d = nc.dram_tensor("x", [BS, T], mdt, kind="ExternalInput")
    md_d = nc.dram_tensor("metadata", [BS, F], f32, kind="ExternalInput")
    y_d = nc.dram_tensor("y", [BS, T], f32, kind="ExternalOutput")
    wd = {name: nc.dram_tensor(name, shp, mdt if name in _MM_STAT else f32,
                               kind="ExternalInput")
          for name, shp in _WSHAPES.items()}

    with TileContext(nc) as tc:
        with tc.tile_pool(name="const", bufs=1) as cp, \
             tc.tile_pool(name="scr", bufs=1, space="DRAM") as dp:

            # ---- constants to SBUF
            cw = {}
            for name, shp in _WSHAPES.items():
                t = cp.tile(shp, mdt if name in _MM_STAT else f32,
                            name=f"c_{name}")
                nc.sync.dma_start(out=t[:], in_=wd[name][:])
                cw[name] = t
            eps_s = cp.tile([128, 1], f32, name="eps_s")
            nc.vector.memset(eps_s[:], 1e-12)

            # ---- style MLP (per-core 32 samples)
            with tc.tile_pool(name="stp", bufs=1, space="PSUM") as sp:
                mdT = cp.tile([F, BS], f32, name="mdT")
                nc.sync.dma_start(out=mdT[:], in_=md_d.rearrange("s f -> f s"))
                ps1 = sp.tile([64, BS], f32, name="ps1")
                nc.tensor.matmul(ps1[:], cw["mw1"][:], mdT[:],
                                 start=True, stop=True)
                s1 = cp.tile([64, BS], f32, name="s1")
                nc.scalar.activation(s1[:], ps1[:], AF.Relu, bias=cw["mb1c"][:])
                ps2 = sp.tile([128, BS], f32, name="ps2")
                nc.tensor.matmul(ps2[:], cw["mw2"][:], s1[:],
                                 start=True, stop=True)
                s2 = cp.tile([128, BS], f32, name="s2")
                nc.scalar.activation(s2[:], ps2[:], AF.Relu, bias=cw["mb2c"][:])
                ps3 = sp.tile([64, BS], f32, name="ps3")
                nc.tensor.matmul(ps3[:], cw["mw3"][:], s2[:],
                                 start=True, stop=True)
                sT = cp.tile([64, BS], f32, name="sT")
                nc.scalar.activation(sT[:], ps3[:], AF.Identity,
                                     bias=cw["mb3c"][:])

            # ---- per-(layer, supergroup) scale/bias vectors via DRAM trip
            # sT row = 16(l-1) + 2c + (0 scale / 1 bias), col = 16 sg + s'
            sT_d = dp.tile([64, BS], f32, name="sT_d")
            nc.gpsimd.dma_start(out=sT_d[:], in_=sT[:])
            scv = cp.tile([128, 8], f32, name="scv")   # scale, col j=(l-1)*2+sg
            bcv = cp.tile([128, 8], f32, name="bcv")   # bias
            for l in range(1, 5):
                for g in range(2):
                    j = (l - 1) * 2 + g
                    src_s = AP(tensor=sT_d[:].tensor,
                               offset=512 * (l - 1) + 16 * g,
                               ap=((1, 16), (64, 8)))
                    nc.gpsimd.dma_start(out=scv[:, j:j + 1], in_=src_s)
                    src_b = AP(tensor=sT_d[:].tensor,
                               offset=512 * (l - 1) + 32 + 16 * g,
                               ap=((1, 16), (64, 8)))
                    nc.gpsimd.dma_start(out=bcv[:, j:j + 1], in_=src_b)

            # ---------------- main loop
            # Two PSUM pools: "pd" holds d-tiles (+ L5 h5), "pv" holds var
            # tiles; bufs=2 each (8 banks total) so consecutive blocks
            # overlap.  Each 2048-position block is processed as two
            # independent 1024-position streams (c=0,1) that alternate
            # engines: c0 extracts on ACT / prelus on DVE, c1 the opposite.
            with tc.tile_pool(name="pd", bufs=3, space="PSUM") as pdp, \
                 tc.tile_pool(name="pv", bufs=2, space="PSUM") as pvp, \
                 tc.tile_pool(name="xin", bufs=5) as xp, \
                 tc.tile_pool(name="dcpp", bufs=4) as dcpp, \
                 tc.tile_pool(name="dsqp", bufs=3) as dqp, \
                 tc.tile_pool(name="rpk", bufs=3) as rkp, \
                 tc.tile_pool(name="rbc", bufs=3) as rpp, \
                 tc.tile_pool(name="qp", bufs=3) as qpp, \
                 tc.tile_pool(name="ap", bufs=4) as app, \
                 tc.tile_pool(name="outp", bufs=3) as opp:

                def make_stream(g, k):
                    """Return the list of phase closures for block (g, k).

                    27 phases: [xt] + 4 layers x [dmm, extract, vmm, r, q,
                    prelu] + [L5 mm, out].  State is shared via `st`.
                    """
                    st = {}
                    phases = []

                    def p_xt():
                        xt = xp.tile([17, 2048], mdt, name="xt", tag="xt")
                        nc.sync.dma_start(
                            out=xt[0:16, :],
                            in_=x_d[16 * g:16 * g + 16,
                                    2048 * k:2048 * (k + 1)])
                        nc.sync.dma_start(out=xt[16:17, :],
                                          in_=cw["onesr"][:])
                        st["xt"] = xt
                    phases.append(p_xt)

                    def mk_dmm(l):
                        def p_dmm():
                            dts = [pdp.tile([128, 1024], f32,
                                            name=f"dt{l}{c}", tag="pt")
                                   for c in range(2)]
                            src = st["xt"] if l == 1 else st["a"]
                            wname = "w1aug" if l == 1 else f"wb{l}"
                            for tau in range(4):
                                dst = dts[tau // 2][:, 512 * (tau % 2):
                                                    512 * (tau % 2) + 512]
                                sl = slice(512 * tau, 512 * (tau + 1))
                                nc.tensor.matmul(dst, cw[wname][:],
                                                 src[:, sl],
                                                 start=True, stop=True)
                            st["dts"] = dts
                        return p_dmm

                    def mk_ext(l):
                        def p_ext():
                            dts = st["dts"]
                            dcp = dcpp.tile([128, 2048], f16,
                                            name=f"dcp{l}", tag="dcp")
                            bias_ap = (None if l == 1
                                       else cw[f"bv{l}"][:, 0:1])
                            # c0 on ACT
                            if bias_ap is None:
                                nc.scalar.activation(dcp[:, 0:1024],
                                                     dts[0][:], AF.Identity)
                            else:
                                nc.scalar.activation(dcp[:, 0:1024],
                                                     dts[0][:], AF.Identity,
                                                     bias=bias_ap)
                            # c1 on DVE
                            nc.vector.tensor_scalar(
                                dcp[:, 1024:2048], dts[1][:],
                                0.0 if bias_ap is None else bias_ap,
                                None, OP.add)
                            st["dcp"] = dcp
                        return p_ext

                    def mk_dsq(l):
                        def p_dsq():
                            dcp = st["dcp"]
                            dsq = dqp.tile([128, 2048], f16,
                                           name=f"dsq{l}", tag="dsq")
                            nc.vector.tensor_tensor(dsq[:], dcp[:], dcp[:],
                                                    OP.mult)
                            st["dsq"] = dsq
                        return p_dsq

                    def mk_vmm(l):
                        def p_vmm():
                            dsq = st["dsq"]
                            # packed: vbp[32*tau+s, n] = sum_c dsq[8s+c, ...]
                            vbp = pvp.tile([128, 512], f32,
                                           name=f"vb{l}", tag="pv")
                            for tau in range(4):
                                sl = slice(512 * tau, 512 * (tau + 1))
                                nc.tensor.matmul(
                                    vbp[32 * tau:32 * tau + 32, :],
                                    cw["ones32"][:], dsq[:, sl],
                                    start=True, stop=True,
                                    tile_position=(0, 32 * tau))
                            st["vbp"] = vbp
                        return p_vmm

                    def mk_r(l):
                        def p_r():
                            # packed rsqrt, then PE-broadcast to the full
                            # [128, 2048] layout in PSUM
                            rp_s = rkp.tile([128, 512], f16,
                                            name=f"rp{l}", tag="rp")
                            nc.scalar.activation(
                                rp_s[:], st["vbp"][:],
                                AF.Abs_reciprocal_sqrt,
                                scale=1.0 / 7.0, bias=cw["epsv"][:, 0:1])
                            st["rp"] = rp_s
                        return p_r

                    def mk_rbc(l):
                        def p_rbc():
                            rbs = [pdp.tile([128, 1024], f32,
                                            name=f"rb{l}{c}", tag="pt")
                                   for c in range(2)]
                            for tau in range(4):
                                dst = rbs[tau // 2][:, 512 * (tau % 2):
                                                    512 * (tau % 2) + 512]
                                nc.tensor.matmul(
                                    dst,
                                    cw["ebc"][:, 128 * tau:128 * (tau + 1)],
                                    st["rp"][:], start=True, stop=True)
                            st["rbs"] = rbs
                        return p_rbc

                    def mk_q(l):
                        j = (l - 1) * 2 + g

                        def p_q():
                            # z = (dcp * scv) * r   (prelu bias added by ACT)
                            q_ = qpp.tile([128, 2048], f16,
                                          name=f"q{l}", tag="q")
                            for c in range(2):
                                csl = slice(1024 * c, 1024 * (c + 1))
                                nc.vector.scalar_tensor_tensor(
                                    q_[:, csl], st["dcp"][:, csl],
                                    scv[:, j:j + 1], st["rbs"][c][:],
                                    OP.mult, OP.mult)
                            st["q"] = q_
                        return p_q

                    def mk_prelu(l):
                        j = (l - 1) * 2 + g

                        def p_prelu():
                            q_ = st["q"]
                            anew = app.tile([128, 2048], mdt,
                                            name=f"a{l}", tag="a")
                            nc.scalar.activation(
                                anew[:], q_[:], AF.Prelu,
                                bias=bcv[:, j:j + 1], alpha=0.01)
                            st["a"] = anew
                        return p_prelu

                    for l in range(1, 5):
                        phases += [mk_dmm(l), mk_ext(l), mk_dsq(l),
                                   mk_vmm(l), mk_r(l), mk_rbc(l),
                                   mk_q(l), mk_prelu(l)]

                    def p_l5mm():
                        h5t = pdp.tile([128, 1024], f32, name="h5",
                                       tag="pt")
                        h5 = h5t[0:64, 0:512]
                        a4 = st["a"]
                        for tau in range(4):
                            sl = slice(512 * tau, 512 * (tau + 1))
                            nc.tensor.matmul(
                                h5, cw["w5b"][:, 64 * tau:64 * (tau + 1)],
                                a4[:, sl],
                                start=(tau == 0), stop=(tau == 3))
                        st["h5"] = h5
                    phases.append(p_l5mm)

                    def p_out():
                        oc = opp.tile([64, 512], f32, name="oc", tag="oc")
                        nc.scalar.activation(oc[:], st["h5"], AF.Prelu,
                                             bias=cw["b5c"][:], alpha=0.01)
                        ydst = y_d.rearrange(
                            "(sg sp) (kk tau n) -> sg kk tau sp n",
                            sg=2, kk=4, tau=4, n=512)[g, k]
                        # oc partition-major order (p = 16 tau + sp) matches
                        # the (tau, sp, n) iteration of ydst
                        nc.sync.dma_start(out=ydst, in_=oc[:])
                    phases.append(p_out)
                    return phases

                # software-pipelined wavefront emission: streams start
                # STAG phases apart; most-advanced stream emitted first
                streams = [make_stream(g, k)
                           for g in range(2) for k in range(4)]
                STAG = int(cfg.get("stag", 7))
                nph = len(streams[0])
                total = STAG * (len(streams) - 1) + nph
                for t in range(total):
                    for s, phases in enumerate(streams):
                        p = t - STAG * s
                        if 0 <= p < nph:
                            phases[p]()

    nc.compile()
    return nc


# ------------------------------------------------------------------- runner

def _get_program():
    key = tuple(sorted(CFG.items()))
    if key not in _CACHE:
        _CACHE[key] = build_program(CFG)
    return _CACHE[key]


def _make_in_maps(inputs):
    mm_np = {"fp16": np.float16, "f32": np.float32}[CFG["mm_dt"]]
    x = np.ascontiguousarray(
        np.asarray(inputs["x"], dtype=np.float32).reshape(B_FULL, T).astype(
            mm_np))
    md = np.ascontiguousarray(np.asarray(inputs["metadata"], dtype=np.float32))
    wts = _host_weights(inputs)
    in_maps = []
    for i in range(NCORES):
        m = dict(wts)
        m["x"] = np.ascontiguousarray(x[BS * i:BS * (i + 1)])
        m["metadata"] = np.ascontiguousarray(md[BS * i:BS * (i + 1)])
        in_maps.append(m)
    return in_maps


def run_spmd(inputs, trace=False):
    """Run on all 8 cores; returns (y_full, BassKernelResults)."""
    from concourse.bass_utils import run_bass_kernel_spmd
    nc = _get_program()
    in_maps = _make_in_maps(inputs)
    res = run_bass_kernel_spmd(nc, in_maps, core_ids=list(range(NCORES)),
                               trace=trace)
    y = np.concatenate([np.asarray(r["y"]) for r in res.results], axis=0)
    y = y.reshape(B_FULL, 1, T).astype(np.float32)
    return y, res


def kernel(**inputs):
    y, _ = run_spmd(inputs, trace=False)
    return y
